# revision 1
# baseline (speedup 1.0000x reference)
"""F1-score (histogram_binning) Trainium2 Bass kernel.

Computes: pred = argmax(y_pred, axis=1); cm = confusion_matrix(y_true, pred);
then the scalar F1 epilogue of the reference.

Strategy (data-parallel over samples, 8 cores), engines balanced:
  - Stream y_pred shard in 1MB blocks [128 part(samples) x G=16 groups x 128].
  - VectorE: row-max reduce; is_ge one-hot (one TT) for DVE_GROUPS groups;
    oh_true = (iota == label) as ONE pair-packed bf16 TT (2x_1P mode).
  - ScalarE: Sign(x - max) for the remaining groups -> (oh_pred - 1) in
    {-1, 0}; exact correction recovered on host from row sums + bincount:
    rowsum = hist_all - 128*hist_act  =>  cm[i,j] += hist_act[i].
  - TensorE: cm_psum += oh_true^T @ oh_pred, 1024x 128-contraction matmuls
    accumulating into one PSUM bank.
  - Host: sum 8 partial [128,128] outputs, apply correction, F1 epilogue.

Measured: ~273 us/core HW exec (memory roofline ~179 us at 358 GB/s/core),
F1 bit-exact vs the jax reference.
"""

import sys

import numpy as np

sys.path.insert(0, "/opt/trn_rl_repo")

import ml_dtypes  # noqa: E402

import concourse.bacc as bacc  # noqa: E402
import concourse.bass as bass  # noqa: E402
import concourse.tile as tile  # noqa: E402
from concourse import mybir  # noqa: E402
from concourse.bass_utils import run_bass_kernel_spmd  # noqa: E402

N_CORES = 8
N_SAMPLES = 1048576
C = 128
EPS = 1e-07
N_PER_CORE = N_SAMPLES // N_CORES  # 131072
P = 128  # partitions
F_PER_PART = N_PER_CORE // P  # 1024 samples per partition
G = 16  # sample-groups per block
N_BLOCKS = F_PER_PART // G  # 128
DVE_GROUPS = 5  # groups whose is_ge runs on DVE; rest use ACT Sign path


def build_program():
    nc = bacc.Bacc("TRN2")

    y_pred = nc.dram_tensor(
        "y_pred", [N_PER_CORE, C], mybir.dt.float32, kind="ExternalInput"
    )
    # aux[p, :2*F_PER_PART] = labels duplicated in adjacent pairs (bf16,
    # enables DVE 2x_1P packed reads); then iota 0..C-1; then a 1.0 column.
    AUXW = 2 * F_PER_PART + C + 1
    aux_bf16 = nc.dram_tensor(
        "aux_bf16", [P, AUXW], mybir.dt.bfloat16, kind="ExternalInput"
    )
    out_t = nc.dram_tensor("out", [C, C], mybir.dt.float32, kind="ExternalOutput")

    # blocks whose oh_true is host-precomputed and streamed from HBM
    pre_blocks = [b for b in range(N_BLOCKS) if b % 8 < 5]
    oh_pre_t = nc.dram_tensor(
        "oh_pre", [P, len(pre_blocks), G, C], mybir.dt.bfloat16, kind="ExternalInput"
    )

    # sample s_local = p * F_PER_PART + b*G + g  (each partition owns
    # F_PER_PART consecutive samples -> fully contiguous per-partition DMA)
    xs = y_pred[:].rearrange("(p b g) c -> p b g c", p=P, b=N_BLOCKS, g=G)

    with tile.TileContext(nc) as tc:
        with (
            tc.tile_pool(name="consts", bufs=1) as consts,
            tc.tile_pool(name="xp", bufs=8) as xp,
            tc.tile_pool(name="ohp", bufs=12) as ohp,
            tc.tile_pool(name="small", bufs=8) as small,
            tc.tile_pool(name="psum", bufs=2, space="PSUM") as psum_pool,
            tc.tile_pool(name="outp", bufs=1) as outp,
        ):
            aux_sb = consts.tile([P, AUXW], mybir.dt.bfloat16)
            nc.gpsimd.dma_start(out=aux_sb, in_=aux_bf16[:])
            iota_off = 2 * F_PER_PART
            iota_sl = aux_sb[:, iota_off : iota_off + C]

            cm_psum = psum_pool.tile([C, C], mybir.dt.float32)

            # 4D pair-packed APs (innermost [1,2] bf16 -> DVE 2x_1P mode):
            # iota viewed [P, G(bcast), 64, 2]
            iota_bc = bass.AP(
                tensor=iota_sl.tensor,
                offset=iota_sl.offset,
                ap=[[AUXW, P], [0, G], [2, 64], [1, 2]],
            )

            for b in range(N_BLOCKS):
                x_t = xp.tile([P, G, C], mybir.dt.float32)
                nc.sync.dma_start(out=x_t, in_=xs[:, b])

                rowmax = small.tile([P, G], mybir.dt.float32)
                nc.vector.tensor_reduce(
                    out=rowmax,
                    in_=x_t,
                    axis=mybir.AxisListType.X,
                    op=mybir.AluOpType.max,
                )
                negmax = small.tile([P, G], mybir.dt.float32, tag="negmax")
                nc.vector.tensor_scalar_mul(
                    out=negmax[:, DVE_GROUPS:G],
                    in0=rowmax[:, DVE_GROUPS:G],
                    scalar1=-1.0,
                )

                oh_true_t = ohp.tile([P, G, C], mybir.dt.bfloat16, tag="oht")
                oh = ohp.tile([P, G, C], mybir.dt.bfloat16, tag="ohp")
                if b in pre_blocks:
                    # oh_true streamed pre-built from HBM (spare bandwidth),
                    # contiguous 4KB-per-partition destination
                    nc.sync.dma_start(
                        out=oh_true_t, in_=oh_pre_t[:, pre_blocks.index(b)]
                    )
                else:
                    # oh_true = (iota == label), one pair-packed DVE TT
                    labels_pairs = bass.AP(
                        tensor=aux_sb.tensor,
                        offset=aux_sb.offset + b * G * 2,
                        ap=[[AUXW, P], [2, G], [0, 64], [1, 2]],
                    )
                    oh_true_4d = bass.AP(
                        tensor=oh_true_t.tensor,
                        offset=oh_true_t.offset,
                        ap=[[G * C, P], [C, G], [2, 64], [1, 2]],
                    )
                    nc.vector.tensor_tensor(
                        out=oh_true_4d,
                        in0=iota_bc,
                        in1=labels_pairs,
                        op=mybir.AluOpType.is_equal,
                    )
                # oh_pred = (x >= max) for the DVE share, one TT
                nc.vector.tensor_tensor(
                    out=oh[:, 0:DVE_GROUPS, :],
                    in0=x_t[:, 0:DVE_GROUPS, :],
                    in1=rowmax[:, 0:DVE_GROUPS].to_broadcast([P, DVE_GROUPS, C]),
                    op=mybir.AluOpType.is_ge,
                )
                for g in range(DVE_GROUPS, G):
                    # oh_pred - 1 = Sign(x - max) on ACT ({-1, 0})
                    nc.scalar.activation(
                        out=oh[:, g, :],
                        in_=x_t[:, g, :],
                        func=mybir.ActivationFunctionType.Sign,
                        bias=negmax[:, g : g + 1],
                        scale=1.0,
                    )

                for g in range(G):
                    first = b == 0 and g == 0
                    last = b == N_BLOCKS - 1 and g == G - 1
                    nc.tensor.matmul(
                        cm_psum,
                        lhsT=oh_true_t[:, g, :],
                        rhs=oh[:, g, :],
                        start=first,
                        stop=last,
                    )

            res_sb = outp.tile([C, C], mybir.dt.float32)
            nc.vector.tensor_copy(out=res_sb, in_=cm_psum)
            nc.gpsimd.dma_start(out=out_t[:], in_=res_sb)

    nc.finalize()
    return nc


_PROGRAM = None


def _get_program():
    global _PROGRAM
    if _PROGRAM is None:
        _PROGRAM = build_program()
    return _PROGRAM


def _shard_inputs(y_pred, y_true):
    y_pred = np.ascontiguousarray(np.asarray(y_pred), dtype=np.float32)
    y_true = np.asarray(y_true)
    iota = np.broadcast_to(np.arange(C, dtype=np.float32), (P, C))
    ones = np.ones((P, 1), dtype=np.float32)
    in_maps = []
    for c in range(N_CORES):
        sl = slice(c * N_PER_CORE, (c + 1) * N_PER_CORE)
        labels = y_true[sl].astype(np.float32).reshape(P, F_PER_PART)
        labels2 = np.repeat(labels, 2, axis=1)
        aux = np.concatenate([labels2, iota, ones], axis=1).astype(ml_dtypes.bfloat16)
        pre_blocks = [b for b in range(N_BLOCKS) if b % 8 < 5]
        lab3 = labels.reshape(P, N_BLOCKS, G)[:, pre_blocks, :]
        oh_pre = (lab3[..., None] == np.arange(C, dtype=np.float32)).astype(
            ml_dtypes.bfloat16
        )
        in_maps.append({"y_pred": y_pred[sl], "aux_bf16": aux, "oh_pre": oh_pre})
    return in_maps


def _epilogue(cm):
    cm = cm.astype(np.float32)
    TP = np.diagonal(cm)
    FP = (C - 1) * cm[:, 1] + cm[:, 0]
    FN = (C - 1) * cm[1, :] + cm[0, :]
    eps = np.float32(EPS)
    sensitivity = np.mean(TP / (TP + FN + eps), dtype=np.float32)
    precision = np.mean(TP / (TP + FP + eps), dtype=np.float32)
    f1 = np.float32(2.0) * (precision * sensitivity / (precision + sensitivity + eps))
    return np.asarray(f1, dtype=np.float32)


def run_on_device(y_pred, y_true, **kwargs):
    """Run the bass kernel on 8 cores; returns (cm_total, results_obj)."""
    nc = _get_program()
    y_true = np.asarray(y_true)
    in_maps = _shard_inputs(y_pred, y_true)
    res = run_bass_kernel_spmd(nc, in_maps, core_ids=list(range(N_CORES)), **kwargs)
    cm = np.zeros((C, C), dtype=np.float64)
    for c, r in enumerate(res.results):
        out = r["out"].astype(np.float64)
        # ACT-group samples contributed (oh_pred - 1); recover the exact
        # per-true-class count of those samples from row sums + bincount:
        # rowsum = hist_all - 128 * hist_act  =>  hist_act known exactly.
        sl = slice(c * N_PER_CORE, (c + 1) * N_PER_CORE)
        hist_all = np.bincount(np.asarray(y_true[sl]).astype(np.int64), minlength=C)
        hist_act = np.rint((hist_all - out.sum(axis=1)) / C)
        cm += out + hist_act[:, None]
    return cm, res


def kernel(y_pred, y_true):
    cm, _ = run_on_device(y_pred, y_true)
    return _epilogue(cm)



# revision 3
# speedup vs baseline: 1.5292x; 1.5292x over previous
"""F1-score (histogram_binning) Trainium2 Bass kernel — mask formulation.

The reference F1 epilogue only consumes diag(cm), cm[:,0], cm[:,1],
cm[0,:], cm[1,:] — not the full confusion matrix. Those reduce to three
per-sample boolean masks plus tiny label bincounts:

  match[s] = (x[s, y_true[s]] >= rowmax[s])   <=>  pred == true
  p0[s]    = (x[s, 0]        >= rowmax[s])    <=>  pred == 0   (exact:
             argmax is first-max, so x[s,0]==max always means pred 0)
  p1[s]    = (x[s, 1] >= rowmax[s]) & ~p0[s]  <=>  pred == 1

Device work per core (memory-bound, ~64 MiB y_pred stream):
  - 64 blocks of [128 part x 16 samp x 128 cls] fp32 via one HWDGE queue
  - VectorE: rowmax tensor_reduce per block + two tiny strided is_ge TTs
    (columns 0/1 of each sample row) per block; per 16-block chunk one
    is_ge of host-gathered x_true vs rowmax
  - masks accumulate in SBUF, one 768 KB bf16 store at the end
No one-hots, no matmuls, no ScalarE work: DMA is the only near-saturated
engine. Host: bincounts of y_true over the masks, argmax of the ~16k rows
with true<=1 (rows 0/1 of cm), then the exact fp32 F1 epilogue.
"""

import sys

import numpy as np

sys.path.insert(0, "/opt/trn_rl_repo")

import concourse.bacc as bacc  # noqa: E402
import concourse.tile as tile  # noqa: E402
from concourse import mybir  # noqa: E402
from concourse.bass_utils import run_bass_kernel_spmd  # noqa: E402

N_CORES = 8
N_SAMPLES = 1048576
C = 128
EPS = 1e-07
N_PER_CORE = N_SAMPLES // N_CORES  # 131072
P = 128  # partitions
F_PER_PART = N_PER_CORE // P  # 1024 samples per partition
G = 16  # samples per partition per block
N_BLOCKS = F_PER_PART // G  # 64 blocks of 1 MiB
CHUNK = 16  # blocks per match-TT / 256 samples per partition
N_CHUNKS = N_BLOCKS // CHUNK


def build_program():
    nc = bacc.Bacc("TRN2")

    y_pred = nc.dram_tensor(
        "y_pred", [N_PER_CORE, C], mybir.dt.float32, kind="ExternalInput"
    )
    # x_true[p, t] = y_pred_local[p*1024 + t, y_true[p*1024 + t]] (host gather)
    x_true = nc.dram_tensor(
        "x_true", [P, F_PER_PART], mybir.dt.float32, kind="ExternalInput"
    )
    # masks[p, 0, :]=match, [p, 1, :]=pred0, [p, 2, :]=pred1-ish (x1>=max)
    masks_t = nc.dram_tensor(
        "masks", [P, 3, F_PER_PART], mybir.dt.bfloat16, kind="ExternalOutput"
    )

    # sample s_local = p * F_PER_PART + b*G + g -> contiguous per-partition DMA
    xs = y_pred[:].rearrange("(p b g) c -> p b g c", p=P, b=N_BLOCKS, g=G)

    with tile.TileContext(nc) as tc:
        with (
            tc.tile_pool(name="consts", bufs=1) as consts,
            tc.tile_pool(name="xp", bufs=10) as xp,
        ):
            xt_sb = consts.tile([P, F_PER_PART], mybir.dt.float32, tag="xt")
            nc.gpsimd.dma_start(out=xt_sb, in_=x_true[:])

            rm_all = consts.tile([P, F_PER_PART], mybir.dt.float32, tag="rm")
            mk_all = consts.tile([P, 3, F_PER_PART], mybir.dt.bfloat16, tag="mk")

            for b in range(N_BLOCKS):
                x_t = xp.tile([P, G, C], mybir.dt.float32)
                nc.sync.dma_start(out=x_t, in_=xs[:, b])

                sl = slice(b * G, (b + 1) * G)
                nc.vector.tensor_reduce(
                    out=rm_all[:, sl],
                    in_=x_t,
                    axis=mybir.AxisListType.X,
                    op=mybir.AluOpType.max,
                )
                # pred==0 / pred==1 candidates: strided column reads of x_t
                nc.vector.tensor_tensor(
                    out=mk_all[:, 1, sl],
                    in0=x_t[:, :, 0],
                    in1=rm_all[:, sl],
                    op=mybir.AluOpType.is_ge,
                )
                nc.vector.tensor_tensor(
                    out=mk_all[:, 2, sl],
                    in0=x_t[:, :, 1],
                    in1=rm_all[:, sl],
                    op=mybir.AluOpType.is_ge,
                )
                if b % CHUNK == CHUNK - 1:
                    k = b // CHUNK
                    ck = slice(k * CHUNK * G, (k + 1) * CHUNK * G)
                    nc.vector.tensor_tensor(
                        out=mk_all[:, 0, ck],
                        in0=xt_sb[:, ck],
                        in1=rm_all[:, ck],
                        op=mybir.AluOpType.is_ge,
                    )

            nc.sync.dma_start(out=masks_t[:], in_=mk_all)

    nc.finalize()
    return nc


_PROGRAM = None


def _get_program():
    global _PROGRAM
    if _PROGRAM is None:
        _PROGRAM = build_program()
    return _PROGRAM


def _shard_inputs(y_pred, y_true):
    y_pred = np.ascontiguousarray(np.asarray(y_pred), dtype=np.float32)
    y_true = np.asarray(y_true).astype(np.int64)
    x_true_full = np.take_along_axis(y_pred, y_true[:, None], axis=1)[:, 0]
    in_maps = []
    for c in range(N_CORES):
        sl = slice(c * N_PER_CORE, (c + 1) * N_PER_CORE)
        in_maps.append(
            {
                "y_pred": y_pred[sl],
                "x_true": np.ascontiguousarray(
                    x_true_full[sl].reshape(P, F_PER_PART)
                ),
            }
        )
    return in_maps


def _assemble(y_pred, y_true, match, p0, p1):
    """Exact F1 from masks + tiny host bincounts (validated vs reference)."""
    y_true = np.asarray(y_true).astype(np.int64)
    pred1 = p1 & ~p0  # exact pred==1 even under 0-1 ties
    TP = np.bincount(y_true[match], minlength=C).astype(np.float32)
    col0 = np.bincount(y_true[p0], minlength=C).astype(np.float32)
    col1 = np.bincount(y_true[pred1], minlength=C).astype(np.float32)
    sel = y_true <= 1
    pred_sel = np.argmax(y_pred[sel], axis=1)
    t_sel = y_true[sel]
    row0 = np.bincount(pred_sel[t_sel == 0], minlength=C).astype(np.float32)
    row1 = np.bincount(pred_sel[t_sel == 1], minlength=C).astype(np.float32)

    FP = np.float32(C - 1) * col1 + col0
    FN = np.float32(C - 1) * row1 + row0
    eps = np.float32(EPS)
    sensitivity = np.mean(TP / (TP + FN + eps), dtype=np.float32)
    precision = np.mean(TP / (TP + FP + eps), dtype=np.float32)
    f1 = np.float32(2.0) * (precision * sensitivity / (precision + sensitivity + eps))
    return np.asarray(f1, dtype=np.float32)


def run_on_device(y_pred, y_true, **kwargs):
    """Run the bass kernel on 8 cores; returns (masks_tuple, results_obj)."""
    nc = _get_program()
    y_pred = np.ascontiguousarray(np.asarray(y_pred), dtype=np.float32)
    y_true = np.asarray(y_true)
    in_maps = _shard_inputs(y_pred, y_true)
    res = run_bass_kernel_spmd(nc, in_maps, core_ids=list(range(N_CORES)), **kwargs)
    parts = {0: [], 1: [], 2: []}
    for r in res.results:
        m = np.asarray(r["masks"]).astype(np.float32)  # [P, 3, F_PER_PART]
        for j in range(3):
            parts[j].append(m[:, j, :].reshape(-1))  # s_local = p*1024 + t
    match = np.concatenate(parts[0]) > 0.5
    p0 = np.concatenate(parts[1]) > 0.5
    p1 = np.concatenate(parts[2]) > 0.5
    return (match, p0, p1), res


def kernel(y_pred, y_true):
    y_pred = np.ascontiguousarray(np.asarray(y_pred), dtype=np.float32)
    (match, p0, p1), _ = run_on_device(y_pred, y_true)
    return _assemble(y_pred, y_true, match, p0, p1)
